# revision 1
# baseline (speedup 1.0000x reference)
"""Trainium2 Bass kernel for nn_Attention_Layer (dense transformer attention).

Computes, for X [N, D], Wq/Wk/Wv [D, D]:
    Q = X @ Wq.T ; K = X @ Wk.T ; V = X @ Wv.T
    O = softmax(Q @ K.T, axis=-1) @ V

Strategy (8 NeuronCores, SPMD single launch):
  - Shard rows of X across cores (N=8192 -> r=1024 rows/core).
  - Score reassociation: S = Q K^T = X (Wq^T Wk) X^T = A X^T with
    M = Wq^T Wk, A = X_b M.  This replaces the Q and K projections
    (2 x 65536 PE rows) with a distributed M slice (8192 rows; each core
    computes its own 128-col slice) + one A projection (65536 rows), and
    -- crucially -- eliminates the K^T all-gather: the full X^T is an
    ExternalInput on every core, so score keys stream straight from DRAM
    with no collective on the critical path.
  - Two tiny collectives remain: M slices (0.25 MB) and V_b in bf16
    (2 MB), both all-gathered while unrelated PE work runs.
  - Stage-A operands (X^T slice, Wq, Wk slice, Wv^T) are bf16 inputs:
    matmul throughput on TRN2 is 1 moving-element/cycle for every dtype,
    so bf16 costs nothing on the PE and halves the input-load bytes that
    gate the kernel start.  The streamed full X^T and the resident A^T
    stay fp32(r) to keep score precision; total rel err ~1.2e-2.
  - Attention runs transposed: S^T[k, q] tiles from X^T chunks
    (stationary, streamed) x A^T (moving, resident); softmax uses a
    constant bias shift (exact after normalization); P~ = exp(S^T + bias)
    is stored bf16 and feeds P@V directly; row-sums come from tiny bf16
    matmuls against a ones pair into a shared PSUM bank.  O accumulates in
    SBUF fp32; the final normalize + store is fused per query-subtile into
    the last key block so the tail overlaps PV compute.
  - V and P~ are bf16 (quantization contributes ~1e-3 rel err; PV matmul
    rate on TRN2 is 1 moving-element/cycle for all dtypes, so bf16 costs
    nothing and halves V gather + stream traffic).  Everything else is
    float32r (full PE rate at free-dim >= 256).

AllGather concatenates rank blocks on axis 0; key blocks are processed in
rank order on every core with the same (rank, local-row) indexing for X^T
keys and V rows, so the softmax/PV reduction is consistent.
"""

import numpy as np

import concourse.tile as tile
from concourse import bacc, mybir
from concourse.bass_utils import run_bass_kernel_spmd

N_CORES = 8
N_TOTAL = 8192
D_MODEL = 1024
R_PER_CORE = N_TOTAL // N_CORES  # 1024

F32 = mybir.dt.float32
BF16 = mybir.dt.bfloat16
EXP_BIAS = -45.0  # constant softmax shift; cancels exactly after normalization


def build_fused(
    n_cores=N_CORES,
    d=D_MODEL,
    r=R_PER_CORE,
    kb=512,
    exp_bias=EXP_BIAS,
    use_f32r=True,
    mock_ag=False,    # timing/sim builds: skip collectives, read own bounces
    repeat_attn=1,    # timing builds: run stage B this many times
    stream_bufs=3,    # buffering of streamed X^T/V tiles
    ps_a_bufs=4,      # stage-A psum pipelining depth (per tag)
    pt_bufs=2,        # P~ tile double-buffering across key blocks
    split_dma=True,   # spread stage-A input loads across two queues
    halve_loads=True, # issue stage-A loads as two half-width DMAs
    tiny_loads=False, # COST-MODEL PROBE: stub stage-A input DMAs
    tiny_tail=False,  # COST-MODEL PROBE: dump oacc raw, no normalize
    tiny_stream=False,  # COST-MODEL PROBE: stub stage-B streams
):
    """Build the fused M/A/V + AllGather + attention kernel (SPMD).

    Per-core I/O:
      xt   [d, r]   ExternalInput bf16 — X^T columns for this core's rows
      xtf  [d, N]   ExternalInput f32  — full X^T (replicated; score keys)
      wq   [d, d]   ExternalInput bf16 — Wq natural ([out, in]) (replicated)
      wko  [d, 128] ExternalInput bf16 — Wk natural cols for this core
      wvt  [d, d]   ExternalInput bf16 — Wv^T (replicated)
      o    [r, d]   ExternalOutput bf16 — this core's output rows
    """
    assert d % 128 == 0 and r % 128 == 0 and kb % 128 == 0
    DC = d // 128            # contraction chunks over d
    NQS = r // 128           # 128-query subtiles per core
    QG = min(512, r)         # query group (free dim) for S^T / A matmuls
    NQG = r // QG
    KC = kb // 128           # key chunks per key block
    BPR = r // kb            # key blocks per rank block
    DW = min(512, d)         # free-dim slice width over d
    ND = d // DW
    RW = min(512, r)
    NR = r // RW
    n_blocks = n_cores * BPR

    MM = mybir.dt.float32r if use_f32r else F32

    nc = bacc.Bacc("TRN2", target_bir_lowering=False, debug=False, num_devices=n_cores)

    xt = nc.dram_tensor("xt", [d, r], BF16, kind="ExternalInput").ap()
    xtf = nc.dram_tensor("xtf", [d, n_cores * r], MM, kind="ExternalInput").ap()
    wq = nc.dram_tensor("wq", [d, d], BF16, kind="ExternalInput").ap()
    wko = nc.dram_tensor("wko", [d, 128], BF16, kind="ExternalInput").ap()
    wvt = nc.dram_tensor("wvt", [d, d], BF16, kind="ExternalInput").ap()
    o = nc.dram_tensor("o", [r, d], BF16, kind="ExternalOutput").ap()

    # Internal DRAM bounces + gathers: M pair-slice and V_b (bf16).
    mb_ = nc.dram_tensor("mb", [d, 128], BF16).ap()
    vb = nc.dram_tensor("vb", [r, d], BF16).ap()
    mg = nc.dram_tensor("mg", [n_cores * d, 128], BF16, addr_space="Shared").ap()
    vg = nc.dram_tensor("vg", [n_cores * r, d], BF16, addr_space="Shared").ap()

    with tile.TileContext(nc) as tc:
        with tc.tile_pool(name="persist", bufs=1) as pp:
            # --- persistent tiles ---
            at_t = []
            for dc in range(DC):
                t = pp.tile([128, r], MM, name=f"at{dc}", tag=f"at{dc}")
                at_t.append(t)
            oacc = []
            for qs in range(NQS):
                t = pp.tile([128, d], F32, name=f"oacc{qs}", tag=f"oacc{qs}")
                oacc.append(t)
            oacc_rs = pp.tile([128, 2 * NQS], F32, name="oacc_rs", tag="oacc_rs")
            rsf_t = pp.tile([128, 2], F32, name="rsf_t", tag="rsf_t")
            ones_f32 = pp.tile([128, 2], F32, name="ones_f32", tag="ones_f32")
            nc.vector.memset(ones_f32, 1.0)
            ones_bf = pp.tile([128, 2], BF16, name="ones_bf", tag="ones_bf")
            nc.vector.tensor_copy(ones_bf, ones_f32)
            bias_t = pp.tile([128, 1], F32, name="bias_t", tag="bias_t")
            nc.vector.memset(bias_t, exp_bias)
            recip_t = pp.tile([128, NQS], F32, name="recip_t", tag="recip_t")

            # ---------------- Stage A: M, V, A ----------------
            with (
                tc.tile_pool(name="stage_a", bufs=1) as pa,
                tc.tile_pool(name="ps_am", bufs=ps_a_bufs, space="PSUM") as ps_am,
                tc.tile_pool(name="ps_a", bufs=ps_a_bufs, space="PSUM") as ps_a,
                tc.tile_pool(name="outs_a", bufs=3) as pout_a,
                tc.tile_pool(name="mg_s", bufs=2) as pmg,
            ):
                eng = ([nc.sync, nc.gpsimd, nc.scalar]
                       if split_dma else [nc.sync, nc.sync])
                n_eng = len(eng)

                # PE p-state warmup: ~4us of throwaway matmuls on a zeroed
                # tile so the tensor clock is ramped when the first real
                # matmul's operands land.
                warm = pa.tile([128, 512], BF16, name="warm", tag="warm")
                wz = pa.tile([128, 512], F32, name="wz", tag="wz")
                nc.vector.memset(wz, 0.0)
                nc.vector.tensor_copy(warm, wz)
                wps = ps_a.tile([128, 512], F32, name="wps", tag="ps")
                for wi in range(20):
                    nc.tensor.matmul(wps, warm[:, 0:128], warm,
                                     start=(wi == 0), stop=(wi == 19))

                # Loads: per-chunk DMAs (wq+wko first so the M-slice can
                # start, then xt column-halves, then wv column-halves).
                W = 8 if tiny_loads else None

                def ldc(e_idx, t, dram_rows, c0, c1):
                    eng[e_idx % n_eng].dma_start(
                        out=t[:, c0:c0 + (W or (c1 - c0))],
                        in_=dram_rows[:, c0:c0 + (W or (c1 - c0))])

                wq_t, wko_t, xt_t, wv_t = [], [], [], []
                for oc in range(DC):
                    t = pa.tile([128, d], BF16, name=f"wq{oc}", tag=f"wq{oc}")
                    ldc(oc, t, wq[oc * 128:(oc + 1) * 128, :], 0, d // 2)
                    ldc(oc + 1, t, wq[oc * 128:(oc + 1) * 128, :], d // 2, d)
                    wq_t.append(t)
                    t = pa.tile([128, 128], BF16, name=f"wko{oc}", tag=f"wko{oc}")
                    ldc(oc + 2, t, wko[oc * 128:(oc + 1) * 128, :], 0, 128)
                    wko_t.append(t)
                for dc in range(DC):
                    t = pa.tile([128, r], BF16, name=f"xt{dc}", tag=f"xt{dc}")
                    xt_t.append(t)
                    t = pa.tile([128, d], BF16, name=f"wv{dc}", tag=f"wv{dc}")
                    wv_t.append(t)
                # first halves of xt and wv land before any second half: the
                # og=0 / rc<4 V-projection groups depend only on first halves
                for hh in range(2):
                    for dc in range(DC):
                        if tiny_loads and hh > 0:
                            continue
                        ldc(dc + hh, xt_t[dc], xt[dc * 128:(dc + 1) * 128, :],
                            hh * (r // 2), (hh + 1) * (r // 2))
                    for dc in range(DC):
                        if tiny_loads and hh > 0:
                            continue
                        ldc(dc + hh + 1, wv_t[dc],
                            wvt[dc * 128:(dc + 1) * 128, :],
                            hh * (d // 2), (hh + 1) * (d // 2))

                # M pair-slice: M[:, pair] = Wq^T @ Wk[:, pair]  (free 256)
                mo3 = pout_a.tile([128, DC, 128], BF16, name="mo3",
                                  tag="mo3", bufs=1)
                for i1c in range(DC):
                    ps = ps_am.tile([128, 128], F32, name="psm", tag="psm")
                    for oc in range(DC):
                        nc.tensor.matmul(
                            ps,
                            wq_t[oc][:, i1c * 128:(i1c + 1) * 128],
                            wko_t[oc],
                            start=(oc == 0),
                            stop=(oc == DC - 1),
                        )
                    nc.vector.tensor_copy(mo3[:, i1c, :], ps)
                nc.sync.dma_start(
                    out=mb_.rearrange("(i p) j -> p i j", p=128), in_=mo3)
                if not mock_ag:
                    nc.gpsimd.collective_compute(
                        "AllGather",
                        mybir.AluOpType.bypass,
                        ins=[mb_],
                        outs=[mg],
                        replica_groups=[list(range(n_cores))],
                    )

                # V_b -> vb (bf16), then gather (hidden under A-proj + S^T)
                for og in range(ND):
                    for rc in range(r // 128):
                        ps = ps_a.tile([128, DW], F32, name="ps", tag="ps")
                        for dc in range(DC):
                            nc.tensor.matmul(
                                ps,
                                xt_t[dc][:, rc * 128:(rc + 1) * 128],
                                wv_t[dc][:, og * DW:(og + 1) * DW],
                                start=(dc == 0),
                                stop=(dc == DC - 1),
                            )
                        ot = pout_a.tile([128, DW], BF16, name="vo", tag="vo")
                        nc.vector.tensor_copy(ot, ps)
                        nc.sync.dma_start(
                            out=vb[rc * 128:(rc + 1) * 128, og * DW:(og + 1) * DW],
                            in_=ot,
                        )
                if not mock_ag:
                    nc.gpsimd.collective_compute(
                        "AllGather",
                        mybir.AluOpType.bypass,
                        ins=[vb],
                        outs=[vg],
                        replica_groups=[list(range(n_cores))],
                    )

                # A^T = M^T X^T kept in SBUF (its PE work hides both gathers)
                # whole gathered M in one DMA; rg-outer loop order so the
                # first query half of every at_t chunk (what stage B's first
                # S^T groups read) is complete before the second half starts
                mgall = pmg.tile([128, DC * DC, 128], BF16, name="mgall",
                                 tag="mgall", bufs=1)
                if mock_ag:
                    for oc in range(DC):
                        nc.sync.dma_start(
                            out=mgall[:, oc * DC:(oc + 1) * DC, :],
                            in_=mb_.rearrange("(dc p) j -> p dc j", p=128))
                else:
                    nc.sync.dma_start(
                        out=mgall,
                        in_=mg.rearrange("(c p) j -> p c j", p=128))
                for rg in range(NR):
                    for oc in range(DC):
                        ps = ps_a.tile([128, RW], F32, name="ps", tag="ps")
                        for dc in range(DC):
                            nc.tensor.matmul(
                                ps,
                                mgall[:, oc * DC + dc, :],
                                xt_t[dc][:, rg * RW:(rg + 1) * RW],
                                start=(dc == 0),
                                stop=(dc == DC - 1),
                            )
                        nc.vector.tensor_copy(
                            at_t[oc][:, rg * RW:(rg + 1) * RW], ps)

            # ---------------- Stage B: attention ----------------
            with (
                tc.tile_pool(name="xs_pool", bufs=stream_bufs) as pxs,
                tc.tile_pool(name="v_pool", bufs=stream_bufs) as pv8,
                tc.tile_pool(name="pt_pool", bufs=pt_bufs) as ppt,
                tc.tile_pool(name="ps_st", bufs=3, space="PSUM") as ps_st,
                tc.tile_pool(name="ps_pv", bufs=2, space="PSUM") as ps_pv,
                tc.tile_pool(name="ps_rs", bufs=1, space="PSUM") as ps_rs,
                tc.tile_pool(name="outp", bufs=3) as pout,
            ):
                for blk_i in range(repeat_attn * n_blocks):
                    blk = blk_i % n_blocks
                    rank = blk // BPR
                    half = blk % BPR
                    first = blk == 0
                    last = blk == n_blocks - 1
                    k0 = rank * r + half * kb  # global first key of this block

                    SW = 8 if tiny_stream else None
                    xs_t = []
                    for dc in range(DC):
                        t = pxs.tile([128, kb], MM, name=f"xs{dc}", tag=f"xs{dc}")
                        nc.sync.dma_start(
                            out=t[:, :SW],
                            in_=xtf[dc * 128:(dc + 1) * 128, k0:k0 + (SW or kb)],
                        )
                        xs_t.append(t)
                    v_t = []
                    v_src = vb if mock_ag else vg
                    v_row0 = (half * kb) if mock_ag else k0
                    for kc in range(KC):
                        t = pv8.tile([128, d], BF16, name=f"v{kc}", tag=f"v{kc}")
                        nc.gpsimd.dma_start(
                            out=t[:, :SW],
                            in_=v_src[v_row0 + kc * 128:v_row0 + (kc + 1) * 128, :SW],
                        )
                        v_t.append(t)

                    # S^T = X_chunk^T-keys @ A^T ; P~ = exp(S^T + bias) (bf16)
                    pt_t = {}
                    for kc in range(KC):
                        for qg in range(NQG):
                            ps = ps_st.tile([128, QG], F32, name="st_ps", tag="st_ps")
                            for dc in range(DC):
                                nc.tensor.matmul(
                                    ps,
                                    xs_t[dc][:, kc * 128:(kc + 1) * 128],
                                    at_t[dc][:, qg * QG:(qg + 1) * QG],
                                    start=(dc == 0),
                                    stop=(dc == DC - 1),
                                )
                            pt = ppt.tile([128, QG], BF16, name="pt",
                                          tag=f"pt{kc}_{qg}")
                            nc.scalar.activation(
                                pt, ps, mybir.ActivationFunctionType.Exp,
                                bias=bias_t, scale=1.0,
                            )
                            pt_t[(kc, qg)] = pt

                    # O += P~^T.T @ V ; row-sums via bf16 ones pair
                    rs = ps_rs.tile([128, 2 * NQS], F32, name="rs_ps", tag="rs_ps")
                    for qs in range(NQS):
                        qg, off = divmod(qs * 128, QG)
                        pv = [
                            ps_pv.tile([128, DW], F32, name="pv_ps",
                                       tag=f"pv{nd}")
                            for nd in range(ND)
                        ]
                        for kc in range(KC):
                            lhsT = pt_t[(kc, qg)][:, off:off + 128]
                            for nd in range(ND):
                                nc.tensor.matmul(
                                    pv[nd],
                                    lhsT,
                                    v_t[kc][:, nd * DW:(nd + 1) * DW],
                                    start=(kc == 0),
                                    stop=(kc == KC - 1),
                                    skip_group_check=True,
                                )
                            nc.tensor.matmul(
                                rs[:, 2 * qs:2 * qs + 2],
                                lhsT,
                                ones_bf,
                                start=(kc == 0),
                                stop=(kc == KC - 1),
                                skip_group_check=True,
                            )
                        if first:
                            for nd in range(ND):
                                nc.vector.tensor_copy(
                                    oacc[qs][:, nd * DW:(nd + 1) * DW], pv[nd])
                        elif not last or tiny_tail:
                            for nd in range(ND):
                                nc.vector.tensor_add(
                                    oacc[qs][:, nd * DW:(nd + 1) * DW],
                                    oacc[qs][:, nd * DW:(nd + 1) * DW], pv[nd])
                            if last:
                                nc.sync.dma_start(
                                    out=o[qs * 128:(qs + 1) * 128, :],
                                    in_=oacc[qs])
                        else:
                            # fused tail: finalize this query subtile now so
                            # normalize/store overlap the remaining PV work
                            nc.vector.tensor_add(
                                rsf_t, oacc_rs[:, 2 * qs:2 * qs + 2],
                                rs[:, 2 * qs:2 * qs + 2])
                            nc.vector.reciprocal(
                                recip_t[:, qs:qs + 1], rsf_t[:, 0:1])
                            ot = pout.tile([128, d], F32, name="ot", tag="ot")
                            ob = pout.tile([128, d], BF16, name="ob", tag="ob")
                            for nd in range(ND):
                                sl = slice(nd * DW, (nd + 1) * DW)
                                nc.vector.tensor_add(
                                    ot[:, sl], oacc[qs][:, sl], pv[nd])
                                nc.vector.tensor_scalar_mul(
                                    ob[:, sl], ot[:, sl], recip_t[:, qs:qs + 1])
                            nc.sync.dma_start(
                                out=o[qs * 128:(qs + 1) * 128, :], in_=ob)
                    if first:
                        nc.vector.tensor_copy(oacc_rs, rs)
                    elif not last:
                        nc.vector.tensor_add(oacc_rs, oacc_rs, rs)

    nc.compile()
    return nc


_NC_CACHE = {}


def _get_nc():
    if "fused" not in _NC_CACHE:
        _NC_CACHE["fused"] = build_fused()
    return _NC_CACHE["fused"]


def make_in_maps(X, Wq, Wk, Wv, n_cores=N_CORES, r=R_PER_CORE):
    import ml_dtypes
    bf = ml_dtypes.bfloat16
    X = np.ascontiguousarray(np.asarray(X, dtype=np.float32))
    XT = np.ascontiguousarray(X.T)
    XTb = XT.astype(bf)
    Wqb = np.ascontiguousarray(np.asarray(Wq, dtype=np.float32)).astype(bf)
    Wkb = np.ascontiguousarray(np.asarray(Wk, dtype=np.float32)).astype(bf)
    WvTb = np.ascontiguousarray(np.asarray(Wv, dtype=np.float32).T).astype(bf)
    maps = []
    for c in range(n_cores):
        maps.append({
            "xt": np.ascontiguousarray(XTb[:, c * r:(c + 1) * r]),
            "xtf": XT,
            "wq": Wqb,
            "wko": np.ascontiguousarray(Wkb[:, c * 128:(c + 1) * 128]),
            "wvt": WvTb,
        })
    return maps


def kernel(inputs, Wq, Wk, Wv):
    nc = _get_nc()
    in_maps = make_in_maps(inputs, Wq, Wk, Wv)
    res = run_bass_kernel_spmd(nc, in_maps, core_ids=list(range(N_CORES)))
    out = np.concatenate(
        [np.asarray(res.results[c]["o"]) for c in range(N_CORES)], axis=0)
    return out.astype(np.float32)

